# revision 1
# baseline (speedup 1.0000x reference)
"""Trainium2 Bass kernel: causal multi-head attention (B=2, N=2048, C=2048, 16 heads).

Sharding: 16 heads split across 8 cores (2 heads/core, tensor parallel).
Each core computes q/k/v projections for its 2 heads, causal attention,
and its partial out-projection y_c = ctx_c @ wo_c.T. Host sums partials + bo.

Per-core layout trick: everything is computed "transposed":
  qT/kT: [head_dim(128) partitions, tokens]  (from lhsT=w^T, rhs=x^T)
  S^T[k, q] = K^T.T @ Q^T tiles              (contraction over head_dim)
  E^T = exp(scale * S^T) (no max subtraction -- scores are ~N(0, 1/9))
  row sums via ones-column matmul (reduces over partitions = k)
  ctx^T[d, q] = V.T @ E^T  (lhsT = V natural [tok, d])
  normalize ctx^T by broadcasting 1/sums across partitions (PE broadcast)
  y[tok, f] = ctx^T.T @ wo^T  (natural output layout)
Causality at tile granularity: only k-tiles with k <= q_max computed;
diagonal tiles multiplied by precomputed 0/1 masks after exp.
"""

import os
import numpy as np

import concourse.bass as bass
import concourse.tile as tile
from concourse import bacc, mybir
from concourse import bass_utils

F32 = mybir.dt.float32
AF = mybir.ActivationFunctionType

# problem dims (hardcoded per contract)
B = 2
N = 2048
C = 2048
HEADS = 16
HD = 128          # head dim
NCORES = 8
HPC = HEADS // NCORES  # heads per core = 2
E = HPC * HD      # per-core projection width = 256
BN = B * N        # 4096
P = 128
CT = C // P       # 16 contraction tiles
NCH = 512         # n-chunk width for projections
NCHUNKS = BN // NCH   # 8
QCW = 512         # q-chunk width in attention
QCHUNKS = N // QCW    # 4 per batch
KT_PER_B = N // P     # 16 k-tiles per batch
TOK_TILES = BN // P   # 32
SCALE = float(HD) ** -0.5

_CACHE = {}


def _build():
    nc = bacc.Bacc(
        "TRN2",
        target_bir_lowering=False,
        debug=False,
        enable_asserts=False,
        num_devices=NCORES,
    )

    xT = nc.dram_tensor("xT", [C, BN], F32, kind="ExternalInput").ap()
    wqT = nc.dram_tensor("wqT", [C, E], F32, kind="ExternalInput").ap()
    wkT = nc.dram_tensor("wkT", [C, E], F32, kind="ExternalInput").ap()
    wvT = nc.dram_tensor("wvT", [C, E], F32, kind="ExternalInput").ap()
    woT = nc.dram_tensor("woT", [E, C], F32, kind="ExternalInput").ap()
    bqh = nc.dram_tensor("bqh", [HPC, P], F32, kind="ExternalInput").ap()
    bkh = nc.dram_tensor("bkh", [HPC, P], F32, kind="ExternalInput").ap()
    bvb = nc.dram_tensor("bvb", [P, E], F32, kind="ExternalInput").ap()
    masks = nc.dram_tensor("masks", [4, P, QCW], F32, kind="ExternalInput").ap()
    yp = nc.dram_tensor("yp", [BN, C], F32, kind="ExternalOutput").ap()

    with tile.TileContext(nc) as tc:
        with tc.tile_pool(name="persist", bufs=1) as persist:
            # persistent per-core activations
            qT = persist.tile([P, HPC, B, N], F32, tag="qT")
            kT = persist.tile([P, HPC, B, N], F32, tag="kT")
            vN = persist.tile([P, TOK_TILES, E], F32, tag="vN")

            # ---------------- Phase 1: projections ----------------
            with tc.tile_pool(name="p1w", bufs=1) as wpool, \
                 tc.tile_pool(name="p1x", bufs=2) as xpool, \
                 tc.tile_pool(name="p1qk_ps", bufs=4, space="PSUM") as qkps, \
                 tc.tile_pool(name="p1v_ps", bufs=2, space="PSUM") as vps:
                wq_sb = wpool.tile([P, CT, E], F32, tag="wq")
                wk_sb = wpool.tile([P, CT, E], F32, tag="wk")
                wv_sb = wpool.tile([P, CT, E], F32, tag="wv")
                bq_sb = wpool.tile([P, HPC], F32, tag="bq")
                bk_sb = wpool.tile([P, HPC], F32, tag="bk")
                bvb_sb = wpool.tile([P, E], F32, tag="bvb")
                nc.sync.dma_start(wq_sb[:], wqT.rearrange("(t p) e -> p t e", p=P))
                nc.sync.dma_start(wk_sb[:], wkT.rearrange("(t p) e -> p t e", p=P))
                nc.sync.dma_start(wv_sb[:], wvT.rearrange("(t p) e -> p t e", p=P))
                nc.sync.dma_start(bq_sb[:], bqh.rearrange("h p -> p h"))
                nc.sync.dma_start(bk_sb[:], bkh.rearrange("h p -> p h"))
                nc.sync.dma_start(bvb_sb[:], bvb)

                xTr = xT.rearrange("(t p) n -> p t n", p=P)
                for ch in range(NCHUNKS):
                    b = ch // (N // NCH)
                    nn0 = (ch % (N // NCH)) * NCH  # within-batch token offset
                    n0 = ch * NCH                  # global token offset
                    # stream x^T column block in two halves (c-tiles 0-7, 8-15)
                    xh = []
                    for half in range(2):
                        xc = xpool.tile([P, CT // 2, NCH], F32, tag="xc")
                        nc.sync.dma_start(
                            xc[:], xTr[:, half * 8:(half + 1) * 8, n0:n0 + NCH]
                        )
                        xh.append(xc)

                    # q^T and k^T for both heads: [hd, tokens]
                    for (wsb, bsb, dst) in ((wq_sb, bq_sb, qT), (wk_sb, bk_sb, kT)):
                        for h in range(HPC):
                            ps = qkps.tile([P, NCH], F32, tag="qkps")
                            for ct in range(CT):
                                nc.tensor.matmul(
                                    ps[:],
                                    wsb[:, ct, h * HD:(h + 1) * HD],
                                    xh[ct // 8][:, ct % 8, :],
                                    start=(ct == 0),
                                    stop=(ct == CT - 1),
                                )
                            # bias add (per-partition) + PSUM->SBUF
                            nc.scalar.activation(
                                dst[:, h, b, nn0:nn0 + NCH], ps[:],
                                AF.Identity, bias=bsb[:, h:h + 1], scale=1.0,
                            )

                    # v natural: [tokens, d] (lhsT = x^T slice)
                    for ts in range(NCH // P):
                        vp = vps.tile([P, E], F32, tag="vp")
                        for ct in range(CT):
                            nc.tensor.matmul(
                                vp[:],
                                xh[ct // 8][:, ct % 8, ts * P:(ts + 1) * P],
                                wv_sb[:, ct, :],
                                start=(ct == 0),
                                stop=(ct == CT - 1),
                            )
                        nc.vector.tensor_add(
                            vN[:, ch * (NCH // P) + ts, :], vp[:], bvb_sb[:]
                        )

            # ---------------- Phase 2: attention + out-proj ----------------
            with tc.tile_pool(name="p2const", bufs=1) as cpool, \
                 tc.tile_pool(name="p2e", bufs=4) as epool, \
                 tc.tile_pool(name="p2ctx", bufs=4) as ctxpool, \
                 tc.tile_pool(name="p2sm", bufs=4) as smpool, \
                 tc.tile_pool(name="p2y", bufs=2) as ysbpool, \
                 tc.tile_pool(name="p2s_ps", bufs=2, space="PSUM") as spool, \
                 tc.tile_pool(name="p2c_ps", bufs=2, space="PSUM") as cps, \
                 tc.tile_pool(name="p2sum_ps", bufs=1, space="PSUM") as sumps, \
                 tc.tile_pool(name="p2b_ps", bufs=1, space="PSUM") as bps, \
                 tc.tile_pool(name="p2y_ps", bufs=2, space="PSUM") as yps:
                masks_sb = cpool.tile([P, 4, QCW], F32, tag="masks")
                nc.sync.dma_start(masks_sb[:], masks.rearrange("a p n -> p a n"))
                wo_sb = cpool.tile([P, HPC, C], F32, tag="wo")
                nc.sync.dma_start(wo_sb[:], woT.rearrange("(h p) f -> p h f", p=P))
                ones_sb = cpool.tile([P, P], F32, tag="ones")
                nc.gpsimd.memset(ones_sb[:], 1.0)

                for b in range(B):
                    for qc in range(QCHUNKS):
                        nkt = 4 * qc + 4  # causal: k-tiles 0..4qc+3
                        ctx_tiles = []
                        for h in range(HPC):
                            sums_ps = sumps.tile([1, QCW], F32, tag="sums")
                            ctxu_ps = cps.tile([P, QCW], F32, tag="ctxu")
                            for kt in range(nkt):
                                sps = spool.tile([P, QCW], F32, tag="s")
                                nc.tensor.matmul(
                                    sps[:],
                                    kT[:, h, b, kt * P:(kt + 1) * P],
                                    qT[:, h, b, qc * QCW:(qc + 1) * QCW],
                                    start=True, stop=True,
                                )
                                et = epool.tile([P, QCW], F32, tag="e")
                                nc.scalar.activation(
                                    et[:], sps[:], AF.Exp, scale=SCALE
                                )
                                a = kt - 4 * qc
                                if a >= 0:  # diagonal tile: causal mask
                                    nc.vector.tensor_mul(
                                        et[:], et[:], masks_sb[:, a, :]
                                    )
                                nc.tensor.matmul(
                                    ctxu_ps[:],
                                    vN[:, b * KT_PER_B + kt, h * HD:(h + 1) * HD],
                                    et[:],
                                    start=(kt == 0), stop=(kt == nkt - 1),
                                )
                                nc.tensor.matmul(
                                    sums_ps[:],
                                    ones_sb[:, 0:1],
                                    et[:],
                                    start=(kt == 0), stop=(kt == nkt - 1),
                                )
                            recip_sb = smpool.tile([1, QCW], F32, tag="recip")
                            nc.vector.reciprocal(recip_sb[:], sums_ps[:])
                            # broadcast 1/sums across partitions via PE
                            bcast_ps = bps.tile([P, QCW], F32, tag="bcast")
                            nc.tensor.matmul(
                                bcast_ps[:], ones_sb[0:1, :], recip_sb[:],
                                start=True, stop=True,
                            )
                            bcast_sb = smpool.tile([P, QCW], F32, tag="bcast_sb")
                            nc.vector.tensor_copy(bcast_sb[:], bcast_ps[:])
                            ctx = ctxpool.tile([P, QCW], F32, tag="ctx")
                            nc.vector.tensor_mul(ctx[:], ctxu_ps[:], bcast_sb[:])
                            ctx_tiles.append(ctx)

                        # out-projection for this (b, qc) block of tokens
                        for nt in range(QCW // P):
                            y_sb = ysbpool.tile([P, C], F32, tag="ysb")
                            for fc in range(C // 512):
                                y_ps = yps.tile([P, 512], F32, tag="yps")
                                for h in range(HPC):
                                    nc.tensor.matmul(
                                        y_ps[:],
                                        ctx_tiles[h][:, nt * P:(nt + 1) * P],
                                        wo_sb[:, h, fc * 512:(fc + 1) * 512],
                                        start=(h == 0), stop=(h == HPC - 1),
                                    )
                                nc.vector.tensor_copy(
                                    y_sb[:, fc * 512:(fc + 1) * 512], y_ps[:]
                                )
                            row0 = b * N + qc * QCW + nt * P
                            nc.sync.dma_start(yp[row0:row0 + P, :], y_sb[:])

    nc.compile()
    return nc


def _host_prep(x, wq, bq, wk, bk, wv, bv, wo):
    """Build the 8 per-core input maps."""
    x = np.asarray(x, dtype=np.float32)
    xT = np.ascontiguousarray(x.reshape(BN, C).T)  # [C, BN]

    m = np.zeros((4, P, QCW), dtype=np.float32)
    kl = np.arange(P)[:, None]
    ql = np.arange(QCW)[None, :]
    for a in range(4):
        m[a] = (ql >= (P * a + kl)).astype(np.float32)

    in_maps = []
    for c in range(NCORES):
        e0 = c * E
        in_maps.append({
            "xT": xT,
            "wqT": np.ascontiguousarray(np.asarray(wq)[e0:e0 + E, :].T),
            "wkT": np.ascontiguousarray(np.asarray(wk)[e0:e0 + E, :].T),
            "wvT": np.ascontiguousarray(np.asarray(wv)[e0:e0 + E, :].T),
            "woT": np.ascontiguousarray(np.asarray(wo)[:, e0:e0 + E].T),
            "bqh": np.ascontiguousarray(
                np.asarray(bq)[e0:e0 + E].reshape(HPC, P)),
            "bkh": np.ascontiguousarray(
                np.asarray(bk)[e0:e0 + E].reshape(HPC, P)),
            "bvb": np.ascontiguousarray(
                np.broadcast_to(np.asarray(bv)[e0:e0 + E], (P, E))),
            "masks": m,
        })
    return in_maps


def kernel(**inputs):
    if "nc" not in _CACHE:
        _CACHE["nc"] = _build()
    nc = _CACHE["nc"]

    in_maps = _host_prep(
        inputs["x"], inputs["wq"], inputs["bq"], inputs["wk"], inputs["bk"],
        inputs["wv"], inputs["bv"], inputs["wo"],
    )

    res = bass_utils.run_bass_kernel_spmd(
        nc, in_maps, core_ids=list(range(NCORES)),
        trace=bool(os.environ.get("BASS_TRACE")),
    )
    _CACHE["last_result"] = res

    y = np.zeros((BN, C), dtype=np.float64)
    for c in range(NCORES):
        y += res.results[c]["yp"].astype(np.float64)
    y += np.asarray(inputs["bo"], dtype=np.float64)
    return y.astype(np.float32).reshape(B, N, C)


# revision 12
# speedup vs baseline: 2.2286x; 2.2286x over previous
"""Trainium2 Bass kernel: causal multi-head attention (B=2, N=2048, C=2048, 16 heads).

Sharding: 16 heads split across 8 cores (2 heads/core, tensor parallel).
Each core computes q/k/v projections for its 2 heads, causal attention,
and its partial out-projection y_c = ctx_c @ wo_c.T. Host sums partials + bo.

Per-core layout trick: everything is computed "transposed":
  qT/kT: [head_dim(128) partitions, tokens]  (from lhsT=w^T, rhs=x^T)
  S^T[k, q] = K^T.T @ Q^T tiles              (contraction over head_dim)
  E^T = exp(scale * S^T) (no max subtraction -- scores are ~N(0, 1/9))
  row sums via ones-column matmul (reduces over partitions = k)
  ctx^T[d, q] = V.T @ E^T  (lhsT = V natural [tok, d])
  normalize ctx^T by broadcasting 1/sums across partitions (PE broadcast)
  y[tok, f] = ctx^T.T @ wo^T  (natural output layout)
Causality at tile granularity: only k-tiles with k <= q_max computed;
diagonal tiles multiplied by precomputed 0/1 masks after exp.
"""

import os
import numpy as np

import concourse.bass as bass
import concourse.tile as tile
from concourse import bacc, mybir
from concourse import bass_utils

F32 = mybir.dt.float32
F32R = mybir.dt.float32r
AF = mybir.ActivationFunctionType


# problem dims (hardcoded per contract)
B = 2
N = 2048
C = 2048
HEADS = 16
HD = 128          # head dim
NCORES = 8
HPC = HEADS // NCORES  # heads per core = 2
E = HPC * HD      # per-core projection width = 256
BN = B * N        # 4096
P = 128
CT = C // P       # 16 contraction tiles
NCH = 512         # n-chunk width for projections
NCHUNKS = BN // NCH   # 8
QCW = 512         # q-chunk width in attention
QCHUNKS = N // QCW    # 4 per batch
KT_PER_B = N // P     # 16 k-tiles per batch
TOK_TILES = BN // P   # 32
SCALE = float(HD) ** -0.5

_CACHE = {}


def _build():
    nc = bacc.Bacc(
        "TRN2",
        target_bir_lowering=False,
        debug=False,
        enable_asserts=False,
        num_devices=NCORES,
    )

    xT = nc.dram_tensor("xT", [C, BN], F32R, kind="ExternalInput").ap()
    wqT = nc.dram_tensor("wqT", [C, E], F32R, kind="ExternalInput").ap()
    wkT = nc.dram_tensor("wkT", [C, E], F32R, kind="ExternalInput").ap()
    wvT = nc.dram_tensor("wvT", [C, E], F32R, kind="ExternalInput").ap()
    woT = nc.dram_tensor("woT", [E, C], F32R, kind="ExternalInput").ap()
    bqh = nc.dram_tensor("bqh", [HPC, P], F32, kind="ExternalInput").ap()
    bkh = nc.dram_tensor("bkh", [HPC, P], F32, kind="ExternalInput").ap()
    bvb = nc.dram_tensor("bvb", [P, E], F32, kind="ExternalInput").ap()
    masks = nc.dram_tensor("masks", [4, P, QCW], F32R, kind="ExternalInput").ap()
    ones_d = nc.dram_tensor("ones_d", [P, P], F32R, kind="ExternalInput").ap()
    yp = nc.dram_tensor("yp", [BN, C], F32, kind="ExternalOutput").ap()

    with tile.TileContext(nc) as tc:
        with tc.tile_pool(name="persist", bufs=1) as persist:
            # persistent per-core activations
            qT = persist.tile([P, HPC, B, N], F32R, tag="qT")
            kT = persist.tile([P, HPC, B, N], F32R, tag="kT")
            vN = persist.tile([P, TOK_TILES, E], F32R, tag="vN")

            # ---------------- Phase 1: projections ----------------
            with tc.tile_pool(name="p1w", bufs=1) as wpool, \
                 tc.tile_pool(name="p1x", bufs=2) as xpool, \
                 tc.tile_pool(name="p1qk_ps", bufs=4, space="PSUM") as qkps, \
                 tc.tile_pool(name="p1v_ps", bufs=2, space="PSUM") as vps:
                wq_sb = wpool.tile([P, CT, E], F32R, tag="wq")
                wk_sb = wpool.tile([P, CT, E], F32R, tag="wk")
                wv_sb = wpool.tile([P, CT, E], F32R, tag="wv")
                bq_sb = wpool.tile([P, HPC], F32, tag="bq")
                bk_sb = wpool.tile([P, HPC], F32, tag="bk")
                bvb_sb = wpool.tile([P, E], F32, tag="bvb")
                nc.sync.dma_start(wq_sb[:], wqT.rearrange("(t p) e -> p t e", p=P))
                nc.sync.dma_start(wk_sb[:], wkT.rearrange("(t p) e -> p t e", p=P))
                nc.sync.dma_start(wv_sb[:], wvT.rearrange("(t p) e -> p t e", p=P))
                nc.sync.dma_start(bq_sb[:], bqh.rearrange("h p -> p h"))
                nc.sync.dma_start(bk_sb[:], bkh.rearrange("h p -> p h"))
                nc.sync.dma_start(bvb_sb[:], bvb)

                xTr = xT.rearrange("(t p) n -> p t n", p=P)
                for ch in range(NCHUNKS):
                    b = ch // (N // NCH)
                    nn0 = (ch % (N // NCH)) * NCH  # within-batch token offset
                    n0 = ch * NCH                  # global token offset
                    # stream x^T column block in two halves (c-tiles 0-7, 8-15)
                    xh = []
                    for half in range(2):
                        xc = xpool.tile([P, CT // 2, NCH], F32R, tag="xc")
                        nc.sync.dma_start(
                            xc[:], xTr[:, half * 8:(half + 1) * 8, n0:n0 + NCH]
                        )
                        xh.append(xc)

                    # q^T and k^T for both heads: [hd, tokens]
                    for (wsb, bsb, dst) in ((wq_sb, bq_sb, qT), (wk_sb, bk_sb, kT)):
                        for h in range(HPC):
                            ps = qkps.tile([P, NCH], F32, tag="qkps")
                            for ct in range(CT):
                                nc.tensor.matmul(
                                    ps[:],
                                    wsb[:, ct, h * HD:(h + 1) * HD],
                                    xh[ct // 8][:, ct % 8, :],
                                    start=(ct == 0),
                                    stop=(ct == CT - 1),
                                )
                            # bias add (per-partition) + PSUM->SBUF
                            nc.scalar.activation(
                                dst[:, h, b, nn0:nn0 + NCH], ps[:],
                                AF.Identity, bias=bsb[:, h:h + 1], scale=1.0,
                            )

                    # v natural: [tokens, d] (lhsT = x^T slice)
                    for ts in range(NCH // P):
                        vp = vps.tile([P, E], F32, tag="vp")
                        for ct in range(CT):
                            nc.tensor.matmul(
                                vp[:],
                                xh[ct // 8][:, ct % 8, ts * P:(ts + 1) * P],
                                wv_sb[:, ct, :],
                                start=(ct == 0),
                                stop=(ct == CT - 1),
                            )
                        nc.vector.tensor_add(
                            vN[:, ch * (NCH // P) + ts, :], vp[:], bvb_sb[:]
                        )

            # ---------------- Phase 2: attention + out-proj ----------------
            with tc.tile_pool(name="p2const", bufs=1) as cpool, \
                 tc.tile_pool(name="p2e", bufs=4) as epool, \
                 tc.tile_pool(name="p2ctx", bufs=4) as ctxpool, \
                 tc.tile_pool(name="p2sm", bufs=4) as smpool, \
                 tc.tile_pool(name="p2y", bufs=2) as ysbpool, \
                 tc.tile_pool(name="p2s_ps", bufs=2, space="PSUM") as spool, \
                 tc.tile_pool(name="p2c_ps", bufs=2, space="PSUM") as cps, \
                 tc.tile_pool(name="p2sum_ps", bufs=1, space="PSUM") as sumps, \
                 tc.tile_pool(name="p2b_ps", bufs=1, space="PSUM") as bps, \
                 tc.tile_pool(name="p2y_ps", bufs=2, space="PSUM") as yps:
                masks_sb = cpool.tile([P, 4, QCW], F32R, tag="masks")
                nc.sync.dma_start(masks_sb[:], masks.rearrange("a p n -> p a n"))
                wo_sb = cpool.tile([P, HPC, C], F32R, tag="wo")
                nc.sync.dma_start(wo_sb[:], woT.rearrange("(h p) f -> p h f", p=P))
                ones_sb = cpool.tile([P, P], F32R, tag="ones")
                nc.sync.dma_start(ones_sb[:], ones_d)

                for b in range(B):
                    for qc in range(QCHUNKS):
                        nkt = 4 * qc + 4  # causal: k-tiles 0..4qc+3
                        ctx_tiles = []
                        for h in range(HPC):
                            sums_ps = sumps.tile([1, QCW], F32, tag="sums")
                            ctxu_ps = cps.tile([P, QCW], F32, tag="ctxu")
                            for kt in range(nkt):
                                sps = spool.tile([P, QCW], F32, tag="s")
                                nc.tensor.matmul(
                                    sps[:],
                                    kT[:, h, b, kt * P:(kt + 1) * P],
                                    qT[:, h, b, qc * QCW:(qc + 1) * QCW],
                                    start=True, stop=True,
                                )
                                et = epool.tile([P, QCW], F32R, tag="e")
                                nc.scalar.activation(
                                    et[:], sps[:], AF.Exp, scale=SCALE
                                )
                                a = kt - 4 * qc
                                if a >= 0:  # diagonal tile: causal mask
                                    nc.vector.tensor_mul(
                                        et[:], et[:], masks_sb[:, a, :]
                                    )
                                nc.tensor.matmul(
                                    ctxu_ps[:],
                                    vN[:, b * KT_PER_B + kt, h * HD:(h + 1) * HD],
                                    et[:],
                                    start=(kt == 0), stop=(kt == nkt - 1),
                                )
                                nc.tensor.matmul(
                                    sums_ps[:],
                                    ones_sb[:, 0:1],
                                    et[:],
                                    start=(kt == 0), stop=(kt == nkt - 1),
                                )
                            recip_f32 = smpool.tile([1, QCW], F32, tag="recip32")
                            nc.vector.reciprocal(recip_f32[:], sums_ps[:])
                            recip_sb = smpool.tile([1, QCW], F32R, tag="recip")
                            nc.vector.tensor_copy(recip_sb[:], recip_f32[:])
                            # broadcast 1/sums across partitions via PE
                            bcast_ps = bps.tile([P, QCW], F32, tag="bcast")
                            nc.tensor.matmul(
                                bcast_ps[:], ones_sb[0:1, :], recip_sb[:],
                                start=True, stop=True,
                            )
                            bcast_sb = smpool.tile([P, QCW], F32, tag="bcast_sb")
                            nc.vector.tensor_copy(bcast_sb[:], bcast_ps[:])
                            ctx = ctxpool.tile([P, QCW], F32R, tag="ctx")
                            nc.vector.tensor_mul(ctx[:], ctxu_ps[:], bcast_sb[:])
                            ctx_tiles.append(ctx)

                        # out-projection for this (b, qc) block of tokens
                        for nt in range(QCW // P):
                            y_sb = ysbpool.tile([P, C], F32, tag="ysb")
                            for fc in range(C // 512):
                                y_ps = yps.tile([P, 512], F32, tag="yps")
                                for h in range(HPC):
                                    nc.tensor.matmul(
                                        y_ps[:],
                                        ctx_tiles[h][:, nt * P:(nt + 1) * P],
                                        wo_sb[:, h, fc * 512:(fc + 1) * 512],
                                        start=(h == 0), stop=(h == HPC - 1),
                                    )
                                nc.vector.tensor_copy(
                                    y_sb[:, fc * 512:(fc + 1) * 512], y_ps[:]
                                )
                            row0 = b * N + qc * QCW + nt * P
                            nc.sync.dma_start(yp[row0:row0 + P, :], y_sb[:])

    nc.compile()
    return nc


def _host_prep(x, wq, bq, wk, bk, wv, bv, wo):
    """Build the 8 per-core input maps."""
    x = np.asarray(x, dtype=np.float32)
    xT = np.ascontiguousarray(x.reshape(BN, C).T)  # [C, BN]

    m = np.zeros((4, P, QCW), dtype=np.float32)
    kl = np.arange(P)[:, None]
    ql = np.arange(QCW)[None, :]
    for a in range(4):
        m[a] = (ql >= (P * a + kl)).astype(np.float32)

    in_maps = []
    for c in range(NCORES):
        e0 = c * E
        in_maps.append({
            "xT": xT,
            "wqT": np.ascontiguousarray(np.asarray(wq)[e0:e0 + E, :].T),
            "wkT": np.ascontiguousarray(np.asarray(wk)[e0:e0 + E, :].T),
            "wvT": np.ascontiguousarray(np.asarray(wv)[e0:e0 + E, :].T),
            "woT": np.ascontiguousarray(np.asarray(wo)[:, e0:e0 + E].T),
            "bqh": np.ascontiguousarray(
                np.asarray(bq)[e0:e0 + E].reshape(HPC, P)),
            "bkh": np.ascontiguousarray(
                np.asarray(bk)[e0:e0 + E].reshape(HPC, P)),
            "bvb": np.ascontiguousarray(
                np.broadcast_to(np.asarray(bv)[e0:e0 + E], (P, E))),
            "masks": m,
            "ones_d": np.ones((P, P), dtype=np.float32),
        })
    return in_maps


def kernel(**inputs):
    if "nc" not in _CACHE:
        _CACHE["nc"] = _build()
    nc = _CACHE["nc"]

    in_maps = _host_prep(
        inputs["x"], inputs["wq"], inputs["bq"], inputs["wk"], inputs["bk"],
        inputs["wv"], inputs["bv"], inputs["wo"],
    )

    res = bass_utils.run_bass_kernel_spmd(
        nc, in_maps, core_ids=list(range(NCORES)),
        trace=bool(os.environ.get("BASS_TRACE")),
    )
    _CACHE["last_result"] = res

    y = np.zeros((BN, C), dtype=np.float64)
    for c in range(NCORES):
        y += res.results[c]["yp"].astype(np.float64)
    y += np.asarray(inputs["bo"], dtype=np.float64)
    return y.astype(np.float32).reshape(B, N, C)


# revision 13
# speedup vs baseline: 2.7748x; 1.2451x over previous
"""Trainium2 Bass kernel: causal multi-head attention (B=2, N=2048, C=2048, 16 heads).

Sharding: 16 heads split across 8 cores (2 heads/core, tensor parallel).
Each core computes q/k/v projections for its 2 heads, causal attention,
and its partial out-projection y_c = ctx_c @ wo_c.T. Host sums partials + bo.

Per-core layout trick: everything is computed "transposed":
  qT/kT/vT: [head_dim(128) partitions, tokens]  (from lhsT=w^T, rhs=x^T)
  vT is PE-transposed back to V natural [tok, d] for the AV matmul
  S^T[k, q] = K^T.T @ Q^T tiles              (contraction over head_dim)
  E^T = exp(scale * S^T) (no max subtraction -- scores are ~N(0, 1/9))
  E tiles accumulated over k-tiles on DVE; row sums via one ones-column
  matmul per (h, q-chunk); ctx^T normalized by PE-broadcast 1/sums
  ctx^T[d, q] = V.T @ E^T  (lhsT = V natural [tok, d])
  y[tok, f] = ctx^T.T @ wo^T  (natural output layout)
Causality at tile granularity: only k-tiles with k <= q_max computed;
diagonal tiles multiplied by precomputed 0/1 masks after exp.
All matmul operands are float32r (full-rate PE streaming, ~19-bit mantissa).
"""

import os
import numpy as np

import concourse.bass as bass
import concourse.tile as tile
from concourse import bacc, mybir
from concourse import bass_utils

F32 = mybir.dt.float32
F32R = mybir.dt.float32r
AF = mybir.ActivationFunctionType

# problem dims (hardcoded per contract)
B = 2
N = 2048
C = 2048
HEADS = 16
HD = 128          # head dim
NCORES = 8
HPC = HEADS // NCORES  # heads per core = 2
E = HPC * HD      # per-core projection width = 256
BN = B * N        # 4096
P = 128
CT = C // P       # 16 contraction tiles
NCH = 512         # n-chunk width for projections
NCHUNKS = BN // NCH   # 8
QCW = 512         # q-chunk width in attention
QCHUNKS = N // QCW    # 4 per batch
KT_PER_B = N // P     # 16 k-tiles per batch
TOK_TILES = BN // P   # 32
SCALE = float(HD) ** -0.5

_CACHE = {}


def _build():
    nc = bacc.Bacc(
        "TRN2",
        target_bir_lowering=False,
        debug=False,
        enable_asserts=False,
        num_devices=NCORES,
    )

    xT = nc.dram_tensor("xT", [C, BN], F32R, kind="ExternalInput").ap()
    wqT = nc.dram_tensor("wqT", [C, E], F32R, kind="ExternalInput").ap()
    wkT = nc.dram_tensor("wkT", [C, E], F32R, kind="ExternalInput").ap()
    wvT = nc.dram_tensor("wvT", [C, E], F32R, kind="ExternalInput").ap()
    woT = nc.dram_tensor("woT", [E, C], F32R, kind="ExternalInput").ap()
    bqh = nc.dram_tensor("bqh", [HPC, P], F32, kind="ExternalInput").ap()
    bkh = nc.dram_tensor("bkh", [HPC, P], F32, kind="ExternalInput").ap()
    bvh = nc.dram_tensor("bvh", [HPC, P], F32, kind="ExternalInput").ap()
    masks = nc.dram_tensor("masks", [4, P, QCW], F32R, kind="ExternalInput").ap()
    ones_d = nc.dram_tensor("ones_d", [P, P], F32R, kind="ExternalInput").ap()
    ident_d = nc.dram_tensor("ident_d", [P, P], F32, kind="ExternalInput").ap()
    yp = nc.dram_tensor("yp", [BN, C], F32, kind="ExternalOutput").ap()

    XQ = 4  # x streamed in quarters of 4 c-tiles

    with tile.TileContext(nc) as tc:
        with tc.tile_pool(name="persist", bufs=1) as persist:
            # persistent per-core activations
            qT = persist.tile([P, HPC, B, N], F32R, tag="qT")
            kT = persist.tile([P, HPC, B, N], F32R, tag="kT")
            vN = persist.tile([P, TOK_TILES, E], F32R, tag="vN")
            # constants needed in phase 2 -- loaded up front to avoid a stall
            masks_sb = persist.tile([P, 4, QCW], F32R, tag="masks")
            ones_sb = persist.tile([P, P], F32R, tag="ones")
            ident_sb = persist.tile([P, P], F32, tag="ident")
            nc.sync.dma_start(masks_sb[:], masks.rearrange("a p n -> p a n"))
            nc.sync.dma_start(ones_sb[:], ones_d)
            nc.sync.dma_start(ident_sb[:], ident_d)

            # ---------------- Phase 1: projections ----------------
            with tc.tile_pool(name="p1w", bufs=1) as wpool, \
                 tc.tile_pool(name="p1x", bufs=4) as xpool, \
                 tc.tile_pool(name="p1vt", bufs=2) as vtpool, \
                 tc.tile_pool(name="p1qk_ps", bufs=4, space="PSUM") as qkps, \
                 tc.tile_pool(name="p1v_ps", bufs=2, space="PSUM") as vps, \
                 tc.tile_pool(name="p1t_ps", bufs=2, space="PSUM") as tps:
                wq_sb = wpool.tile([P, CT, E], F32R, tag="wq")
                wk_sb = wpool.tile([P, CT, E], F32R, tag="wk")
                wv_sb = wpool.tile([P, CT, E], F32R, tag="wv")
                bq_sb = wpool.tile([P, HPC], F32, tag="bq")
                bk_sb = wpool.tile([P, HPC], F32, tag="bk")
                bv_sb = wpool.tile([P, HPC], F32, tag="bv")
                # split weight loads so the first matmuls start early
                for (dst, src) in ((wq_sb, wqT), (wk_sb, wkT), (wv_sb, wvT)):
                    srcr = src.rearrange("(t p) e -> p t e", p=P)
                    for piece in range(4):
                        nc.sync.dma_start(
                            dst[:, piece * 4:(piece + 1) * 4, :],
                            srcr[:, piece * 4:(piece + 1) * 4, :],
                        )
                nc.sync.dma_start(bq_sb[:], bqh.rearrange("h p -> p h"))
                nc.sync.dma_start(bk_sb[:], bkh.rearrange("h p -> p h"))
                nc.sync.dma_start(bv_sb[:], bvh.rearrange("h p -> p h"))

                xTr = xT.rearrange("(t p) n -> p t n", p=P)
                for ch in range(NCHUNKS):
                    b = ch // (N // NCH)
                    nn0 = (ch % (N // NCH)) * NCH  # within-batch token offset
                    n0 = ch * NCH                  # global token offset
                    # stream x^T column block in four quarters (4 c-tiles each)
                    xh = []
                    for quarter in range(XQ):
                        xc = xpool.tile([P, CT // XQ, NCH], F32R, tag="xc")
                        nc.sync.dma_start(
                            xc[:], xTr[:, quarter * 4:(quarter + 1) * 4,
                                       n0:n0 + NCH]
                        )
                        xh.append(xc)

                    # q^T, k^T, v^T for both heads: [hd, tokens]
                    for (wsb, bsb, dst) in (
                        (wq_sb, bq_sb, qT), (wk_sb, bk_sb, kT),
                        (wv_sb, bv_sb, None),
                    ):
                        for h in range(HPC):
                            if dst is None:
                                ps = vps.tile([P, NCH], F32, tag="vtps")
                            else:
                                ps = qkps.tile([P, NCH], F32, tag="qkps")
                            for ct in range(CT):
                                nc.tensor.matmul(
                                    ps[:],
                                    wsb[:, ct, h * HD:(h + 1) * HD],
                                    xh[ct // XQ][:, ct % XQ, :],
                                    start=(ct == 0),
                                    stop=(ct == CT - 1),
                                )
                            if dst is not None:
                                # bias add (per-partition) + PSUM->SBUF
                                nc.scalar.activation(
                                    dst[:, h, b, nn0:nn0 + NCH], ps[:],
                                    AF.Identity, bias=bsb[:, h:h + 1], scale=1.0,
                                )
                            else:
                                # v^T with bias, then PE-transpose to V natural
                                vt = vtpool.tile([P, NCH], F32, tag="vt")
                                nc.scalar.activation(
                                    vt[:], ps[:],
                                    AF.Identity, bias=bsb[:, h:h + 1], scale=1.0,
                                )
                                for ts in range(NCH // P):
                                    tp = tps.tile([P, P], F32, tag="tp")
                                    nc.tensor.transpose(
                                        tp[:], vt[:, ts * P:(ts + 1) * P],
                                        ident_sb[:],
                                    )
                                    nc.vector.tensor_copy(
                                        vN[:, ch * (NCH // P) + ts,
                                           h * HD:(h + 1) * HD],
                                        tp[:],
                                    )

            # ---------------- Phase 2: attention + out-proj ----------------
            with tc.tile_pool(name="p2const", bufs=1) as cpool, \
                 tc.tile_pool(name="p2e", bufs=4) as epool, \
                 tc.tile_pool(name="p2ctx", bufs=4) as ctxpool, \
                 tc.tile_pool(name="p2sm", bufs=4) as smpool, \
                 tc.tile_pool(name="p2ea", bufs=3) as eapool, \
                 tc.tile_pool(name="p2y", bufs=2) as ysbpool, \
                 tc.tile_pool(name="p2s_ps", bufs=2, space="PSUM") as spool, \
                 tc.tile_pool(name="p2c_ps", bufs=2, space="PSUM") as cps, \
                 tc.tile_pool(name="p2sb_ps", bufs=2, space="PSUM") as sbps, \
                 tc.tile_pool(name="p2y_ps", bufs=2, space="PSUM") as yps:
                wo_sb = cpool.tile([P, HPC, C], F32R, tag="wo")
                nc.sync.dma_start(wo_sb[:], woT.rearrange("(h p) f -> p h f", p=P))

                for b in range(B):
                    for qc in range(QCHUNKS):
                        nkt = 4 * qc + 4  # causal: k-tiles 0..4qc+3
                        ctx_tiles = []
                        for h in range(HPC):
                            ctxu_ps = cps.tile([P, QCW], F32, tag="ctxu")
                            ea = eapool.tile([P, QCW], F32R, tag="ea")
                            for kt in range(nkt):
                                sps = spool.tile([P, QCW], F32, tag="s")
                                nc.tensor.matmul(
                                    sps[:],
                                    kT[:, h, b, kt * P:(kt + 1) * P],
                                    qT[:, h, b, qc * QCW:(qc + 1) * QCW],
                                    start=True, stop=True,
                                )
                                et = epool.tile([P, QCW], F32R, tag="e")
                                nc.scalar.activation(
                                    et[:], sps[:], AF.Exp, scale=SCALE
                                )
                                a = kt - 4 * qc
                                if a >= 0:  # diagonal tile: causal mask
                                    nc.vector.tensor_mul(
                                        et[:], et[:], masks_sb[:, a, :]
                                    )
                                nc.tensor.matmul(
                                    ctxu_ps[:],
                                    vN[:, b * KT_PER_B + kt, h * HD:(h + 1) * HD],
                                    et[:],
                                    start=(kt == 0), stop=(kt == nkt - 1),
                                )
                                # accumulate exp tiles for the row sums (DVE)
                                if kt == 0:
                                    nc.vector.tensor_copy(ea[:], et[:])
                                else:
                                    nc.vector.tensor_add(ea[:], ea[:], et[:])
                            # row sums over k (partition dim) via ones matmul
                            sums_ps = sbps.tile([1, QCW], F32, tag="sumbc")
                            nc.tensor.matmul(
                                sums_ps[:], ones_sb[:, 0:1], ea[:],
                                start=True, stop=True,
                            )
                            recip_f32 = smpool.tile([1, QCW], F32, tag="recip32")
                            nc.vector.reciprocal(recip_f32[:], sums_ps[:])
                            recip_sb = smpool.tile([1, QCW], F32R, tag="recip")
                            nc.vector.tensor_copy(recip_sb[:], recip_f32[:])
                            # broadcast 1/sums across partitions via PE
                            bcast_ps = sbps.tile([P, QCW], F32, tag="sumbc")
                            nc.tensor.matmul(
                                bcast_ps[:], ones_sb[0:1, :], recip_sb[:],
                                start=True, stop=True,
                            )
                            bcast_sb = smpool.tile([P, QCW], F32, tag="bcast_sb")
                            nc.scalar.copy(bcast_sb[:], bcast_ps[:])
                            ctx = ctxpool.tile([P, QCW], F32R, tag="ctx")
                            nc.vector.tensor_mul(ctx[:], ctxu_ps[:], bcast_sb[:])
                            ctx_tiles.append(ctx)

                        # out-projection for this (b, qc) block of tokens
                        for nt in range(QCW // P):
                            y_sb = ysbpool.tile([P, C], F32, tag="ysb")
                            for fc in range(C // 512):
                                y_ps = yps.tile([P, 512], F32, tag="yps")
                                for h in range(HPC):
                                    nc.tensor.matmul(
                                        y_ps[:],
                                        ctx_tiles[h][:, nt * P:(nt + 1) * P],
                                        wo_sb[:, h, fc * 512:(fc + 1) * 512],
                                        start=(h == 0), stop=(h == HPC - 1),
                                    )
                                # alternate copy engine so neither DVE nor ACT
                                # becomes the bottleneck for the copy-back
                                if fc % 2 == 0:
                                    nc.vector.tensor_copy(
                                        y_sb[:, fc * 512:(fc + 1) * 512], y_ps[:]
                                    )
                                else:
                                    nc.scalar.copy(
                                        y_sb[:, fc * 512:(fc + 1) * 512], y_ps[:]
                                    )
                            row0 = b * N + qc * QCW + nt * P
                            nc.sync.dma_start(yp[row0:row0 + P, :], y_sb[:])

    nc.compile()
    return nc


def _host_prep(x, wq, bq, wk, bk, wv, bv, wo):
    """Build the 8 per-core input maps."""
    x = np.asarray(x, dtype=np.float32)
    xT = np.ascontiguousarray(x.reshape(BN, C).T)  # [C, BN]

    m = np.zeros((4, P, QCW), dtype=np.float32)
    kl = np.arange(P)[:, None]
    ql = np.arange(QCW)[None, :]
    for a in range(4):
        m[a] = (ql >= (P * a + kl)).astype(np.float32)

    in_maps = []
    for c in range(NCORES):
        e0 = c * E
        in_maps.append({
            "xT": xT,
            "wqT": np.ascontiguousarray(np.asarray(wq)[e0:e0 + E, :].T),
            "wkT": np.ascontiguousarray(np.asarray(wk)[e0:e0 + E, :].T),
            "wvT": np.ascontiguousarray(np.asarray(wv)[e0:e0 + E, :].T),
            "woT": np.ascontiguousarray(np.asarray(wo)[:, e0:e0 + E].T),
            "bqh": np.ascontiguousarray(
                np.asarray(bq)[e0:e0 + E].reshape(HPC, P)),
            "bkh": np.ascontiguousarray(
                np.asarray(bk)[e0:e0 + E].reshape(HPC, P)),
            "bvh": np.ascontiguousarray(
                np.asarray(bv)[e0:e0 + E].reshape(HPC, P)),
            "masks": m,
            "ones_d": np.ones((P, P), dtype=np.float32),
            "ident_d": np.eye(P, dtype=np.float32),
        })
    return in_maps


def kernel(**inputs):
    if "nc" not in _CACHE:
        _CACHE["nc"] = _build()
    nc = _CACHE["nc"]

    in_maps = _host_prep(
        inputs["x"], inputs["wq"], inputs["bq"], inputs["wk"], inputs["bk"],
        inputs["wv"], inputs["bv"], inputs["wo"],
    )

    res = bass_utils.run_bass_kernel_spmd(
        nc, in_maps, core_ids=list(range(NCORES)),
        trace=bool(os.environ.get("BASS_TRACE")),
    )
    _CACHE["last_result"] = res

    y = np.zeros((BN, C), dtype=np.float64)
    for c in range(NCORES):
        y += res.results[c]["yp"].astype(np.float64)
    y += np.asarray(inputs["bo"], dtype=np.float64)
    return y.astype(np.float32).reshape(B, N, C)


# revision 15
# speedup vs baseline: 3.0275x; 1.0911x over previous
"""Trainium2 Bass kernel: causal multi-head attention (B=2, N=2048, C=2048, 16 heads).

Sharding: 16 heads split across 8 cores (2 heads/core, tensor parallel).
Each core computes q/k/v projections for its 2 heads, causal attention,
and its partial out-projection y_c = ctx_c @ wo_c.T. Host sums partials + bo.

Per-core layout trick: everything is computed "transposed":
  qT/kT/vT: [head_dim(128) partitions, tokens]  (from lhsT=w^T, rhs=x^T)
  vT is PE-transposed back to V natural [tok, d] for the AV matmul
  S^T[k, q] = K^T.T @ Q^T tiles              (contraction over head_dim)
  E^T = exp(scale * S^T) (no max subtraction -- scores are ~N(0, 1/9))
  E tiles accumulated over k-tiles on DVE; a single all-ones matmul
  produces the row sums already broadcast across partitions; wide
  reciprocal + multiply normalizes ctx^T
  ctx^T[d, q] = V.T @ E^T  (lhsT = V natural [tok, d])
  y[tok, f] = ctx^T.T @ wo^T  (natural output layout)
Causality at tile granularity: only k-tiles with k <= q_max computed;
diagonal tiles multiplied by precomputed 0/1 masks after exp.
All matmul operands are float32r (full-rate PE streaming, ~19-bit mantissa).
Phase 1 walks c-tiles in the outer loop so x^T quarters release early and
the next chunk's DMA overlaps compute.
"""

import os
import numpy as np

import concourse.bass as bass
import concourse.tile as tile
from concourse import bacc, mybir
from concourse import bass_utils

F32 = mybir.dt.float32
F32R = mybir.dt.float32r
AF = mybir.ActivationFunctionType

# problem dims (hardcoded per contract)
B = 2
N = 2048
C = 2048
HEADS = 16
HD = 128          # head dim
NCORES = 8
HPC = HEADS // NCORES  # heads per core = 2
E = HPC * HD      # per-core projection width = 256
BN = B * N        # 4096
P = 128
CT = C // P       # 16 contraction tiles
NCH = 512         # n-chunk width for projections
NCHUNKS = BN // NCH   # 8
QCW = 512         # q-chunk width in attention
QCHUNKS = N // QCW    # 4 per batch
KT_PER_B = N // P     # 16 k-tiles per batch
TOK_TILES = BN // P   # 32
SCALE = float(HD) ** -0.5
XQ = 4            # x streamed in quarters of 4 c-tiles

_CACHE = {}


def _build():
    nc = bacc.Bacc(
        "TRN2",
        target_bir_lowering=False,
        debug=False,
        enable_asserts=False,
        num_devices=NCORES,
    )

    xT = nc.dram_tensor("xT", [C, BN], F32R, kind="ExternalInput").ap()
    wqT = nc.dram_tensor("wqT", [C, E], F32R, kind="ExternalInput").ap()
    wkT = nc.dram_tensor("wkT", [C, E], F32R, kind="ExternalInput").ap()
    wvT = nc.dram_tensor("wvT", [C, E], F32R, kind="ExternalInput").ap()
    woT = nc.dram_tensor("woT", [E, C], F32R, kind="ExternalInput").ap()
    bqh = nc.dram_tensor("bqh", [HPC, P], F32, kind="ExternalInput").ap()
    bkh = nc.dram_tensor("bkh", [HPC, P], F32, kind="ExternalInput").ap()
    bvh = nc.dram_tensor("bvh", [HPC, P], F32, kind="ExternalInput").ap()
    masks = nc.dram_tensor("masks", [4, P, QCW], F32R, kind="ExternalInput").ap()
    ones_d = nc.dram_tensor("ones_d", [P, P], F32R, kind="ExternalInput").ap()
    ident_d = nc.dram_tensor("ident_d", [P, P], F32, kind="ExternalInput").ap()
    yp = nc.dram_tensor("yp", [BN, C], F32, kind="ExternalOutput").ap()

    with tile.TileContext(nc) as tc:
        with tc.tile_pool(name="persist", bufs=1) as persist:
            # persistent per-core activations
            qT = persist.tile([P, HPC, B, N], F32R, tag="qT")
            kT = persist.tile([P, HPC, B, N], F32R, tag="kT")
            vN = persist.tile([P, TOK_TILES, E], F32R, tag="vN")
            masks_sb = persist.tile([P, 4, QCW], F32R, tag="masks")
            ones_sb = persist.tile([P, P], F32R, tag="ones")
            ident_sb = persist.tile([P, P], F32, tag="ident")

            # ---------------- Phase 1: projections ----------------
            with tc.tile_pool(name="p1w", bufs=1) as wpool, \
                 tc.tile_pool(name="p1x", bufs=4) as xpool, \
                 tc.tile_pool(name="p1vt", bufs=2) as vtpool, \
                 tc.tile_pool(name="p1_ps", bufs=6, space="PSUM") as pps, \
                 tc.tile_pool(name="p1t_ps", bufs=2, space="PSUM") as tps:
                wq_sb = wpool.tile([P, CT, E], F32R, tag="wq")
                wk_sb = wpool.tile([P, CT, E], F32R, tag="wk")
                wv_sb = wpool.tile([P, CT, E], F32R, tag="wv")
                bq_sb = wpool.tile([P, HPC], F32, tag="bq")
                bk_sb = wpool.tile([P, HPC], F32, tag="bk")
                bv_sb = wpool.tile([P, HPC], F32, tag="bv")

                # DMA priority: first weight pieces + chunk-0 x quarters
                # land first so the PE starts within a few us.
                nc.sync.dma_start(bq_sb[:], bqh.rearrange("h p -> p h"))
                nc.sync.dma_start(bk_sb[:], bkh.rearrange("h p -> p h"))
                nc.sync.dma_start(bv_sb[:], bvh.rearrange("h p -> p h"))
                wsrc = [(wq_sb, wqT), (wk_sb, wkT), (wv_sb, wvT)]
                for (dst, src) in wsrc:
                    srcr = src.rearrange("(t p) e -> p t e", p=P)
                    nc.sync.dma_start(dst[:, 0:4, :], srcr[:, 0:4, :])

                xTr = xT.rearrange("(t p) n -> p t n", p=P)
                xh0 = []
                for quarter in range(XQ):
                    xc = xpool.tile([P, CT // XQ, NCH], F32R, tag="xc")
                    nc.sync.dma_start(
                        xc[:], xTr[:, quarter * 4:(quarter + 1) * 4, 0:NCH])
                    xh0.append(xc)

                for (dst, src) in wsrc:
                    srcr = src.rearrange("(t p) e -> p t e", p=P)
                    for piece in range(1, 4):
                        nc.sync.dma_start(
                            dst[:, piece * 4:(piece + 1) * 4, :],
                            srcr[:, piece * 4:(piece + 1) * 4, :],
                        )
                nc.sync.dma_start(masks_sb[:], masks.rearrange("a p n -> p a n"))
                nc.sync.dma_start(ones_sb[:], ones_d)
                nc.sync.dma_start(ident_sb[:], ident_d)

                for ch in range(NCHUNKS):
                    b = ch // (N // NCH)
                    nn0 = (ch % (N // NCH)) * NCH  # within-batch token offset
                    n0 = ch * NCH                  # global token offset
                    if ch == 0:
                        xh = xh0
                    else:
                        xh = []
                        for quarter in range(XQ):
                            xc = xpool.tile([P, CT // XQ, NCH], F32R, tag="xc")
                            nc.sync.dma_start(
                                xc[:], xTr[:, quarter * 4:(quarter + 1) * 4,
                                           n0:n0 + NCH])
                            xh.append(xc)

                    # 6 accumulators (q/k/v x 2 heads); c-tile outer loop so
                    # each x quarter is released after its 4 c-tiles.
                    accs = [pps.tile([P, NCH], F32, tag="pacc",
                                     name=f"pacc_{ch}_{i}")
                            for i in range(3 * HPC)]
                    for ct in range(CT):
                        xq = xh[ct // XQ][:, ct % XQ, :]
                        for wi, (wsb, _) in enumerate(wsrc):
                            for h in range(HPC):
                                nc.tensor.matmul(
                                    accs[wi * HPC + h][:],
                                    wsb[:, ct, h * HD:(h + 1) * HD],
                                    xq,
                                    start=(ct == 0),
                                    stop=(ct == CT - 1),
                                )

                    for h in range(HPC):
                        nc.scalar.activation(
                            qT[:, h, b, nn0:nn0 + NCH], accs[h][:],
                            AF.Identity, bias=bq_sb[:, h:h + 1], scale=1.0)
                        nc.scalar.activation(
                            kT[:, h, b, nn0:nn0 + NCH], accs[HPC + h][:],
                            AF.Identity, bias=bk_sb[:, h:h + 1], scale=1.0)
                        # v^T with bias, then PE-transpose to V natural
                        vt = vtpool.tile([P, NCH], F32, tag="vt")
                        nc.scalar.activation(
                            vt[:], accs[2 * HPC + h][:],
                            AF.Identity, bias=bv_sb[:, h:h + 1], scale=1.0)
                        for ts in range(NCH // P):
                            tp = tps.tile([P, P], F32, tag="tp")
                            nc.tensor.transpose(
                                tp[:], vt[:, ts * P:(ts + 1) * P], ident_sb[:])
                            nc.vector.tensor_copy(
                                vN[:, ch * (NCH // P) + ts,
                                   h * HD:(h + 1) * HD],
                                tp[:])

            # ---------------- Phase 2: attention + out-proj ----------------
            with tc.tile_pool(name="p2const", bufs=1) as cpool, \
                 tc.tile_pool(name="p2e", bufs=4) as epool, \
                 tc.tile_pool(name="p2ctx", bufs=4) as ctxpool, \
                 tc.tile_pool(name="p2sm", bufs=3) as smpool, \
                 tc.tile_pool(name="p2ea", bufs=3) as eapool, \
                 tc.tile_pool(name="p2y", bufs=2) as ysbpool, \
                 tc.tile_pool(name="p2s_ps", bufs=2, space="PSUM") as spool, \
                 tc.tile_pool(name="p2c_ps", bufs=2, space="PSUM") as cps, \
                 tc.tile_pool(name="p2sb_ps", bufs=2, space="PSUM") as sbps, \
                 tc.tile_pool(name="p2y_ps", bufs=2, space="PSUM") as yps:
                wo_sb = cpool.tile([P, HPC, C], F32R, tag="wo")
                nc.sync.dma_start(wo_sb[:], woT.rearrange("(h p) f -> p h f", p=P))

                for b in range(B):
                    for qc in range(QCHUNKS):
                        nkt = 4 * qc + 4  # causal: k-tiles 0..4qc+3
                        ctx_tiles = []
                        for h in range(HPC):
                            ctxu_ps = cps.tile([P, QCW], F32, tag="ctxu")
                            ea = eapool.tile([P, QCW], F32R, tag="ea")
                            for kt in range(nkt):
                                sps = spool.tile([P, QCW], F32, tag="s")
                                nc.tensor.matmul(
                                    sps[:],
                                    kT[:, h, b, kt * P:(kt + 1) * P],
                                    qT[:, h, b, qc * QCW:(qc + 1) * QCW],
                                    start=True, stop=True,
                                )
                                et = epool.tile([P, QCW], F32R, tag="e")
                                nc.scalar.activation(
                                    et[:], sps[:], AF.Exp, scale=SCALE
                                )
                                a = kt - 4 * qc
                                if a >= 0:  # diagonal tile: causal mask
                                    nc.vector.tensor_mul(
                                        et[:], et[:], masks_sb[:, a, :]
                                    )
                                nc.tensor.matmul(
                                    ctxu_ps[:],
                                    vN[:, b * KT_PER_B + kt, h * HD:(h + 1) * HD],
                                    et[:],
                                    start=(kt == 0), stop=(kt == nkt - 1),
                                )
                                # accumulate exp tiles for the row sums (DVE)
                                if kt == 0:
                                    nc.vector.tensor_copy(ea[:], et[:])
                                else:
                                    nc.vector.tensor_add(ea[:], ea[:], et[:])
                            # all-ones lhsT: rows of out = sums over k,
                            # i.e. reduce + broadcast in one matmul
                            sums_bc = sbps.tile([P, QCW], F32, tag="sumbc")
                            nc.tensor.matmul(
                                sums_bc[:], ones_sb[:], ea[:],
                                start=True, stop=True,
                            )
                            recip_bc = smpool.tile([P, QCW], F32, tag="recipbc")
                            nc.vector.reciprocal(recip_bc[:], sums_bc[:])
                            ctx = ctxpool.tile([P, QCW], F32R, tag="ctx")
                            nc.vector.tensor_mul(ctx[:], ctxu_ps[:], recip_bc[:])
                            ctx_tiles.append(ctx)

                        # out-projection for this (b, qc) block of tokens
                        for nt in range(QCW // P):
                            y_sb = ysbpool.tile([P, C], F32, tag="ysb")
                            for fc in range(C // 512):
                                y_ps = yps.tile([P, 512], F32, tag="yps")
                                for h in range(HPC):
                                    nc.tensor.matmul(
                                        y_ps[:],
                                        ctx_tiles[h][:, nt * P:(nt + 1) * P],
                                        wo_sb[:, h, fc * 512:(fc + 1) * 512],
                                        start=(h == 0), stop=(h == HPC - 1),
                                    )
                                # alternate copy engine so neither DVE nor ACT
                                # becomes the bottleneck for the copy-back
                                if fc % 2 == 0:
                                    nc.vector.tensor_copy(
                                        y_sb[:, fc * 512:(fc + 1) * 512], y_ps[:]
                                    )
                                else:
                                    nc.scalar.copy(
                                        y_sb[:, fc * 512:(fc + 1) * 512], y_ps[:]
                                    )
                            row0 = b * N + qc * QCW + nt * P
                            nc.sync.dma_start(yp[row0:row0 + P, :], y_sb[:])

    nc.compile()
    return nc


def _host_prep(x, wq, bq, wk, bk, wv, bv, wo):
    """Build the 8 per-core input maps."""
    x = np.asarray(x, dtype=np.float32)
    xT = np.ascontiguousarray(x.reshape(BN, C).T)  # [C, BN]

    m = np.zeros((4, P, QCW), dtype=np.float32)
    kl = np.arange(P)[:, None]
    ql = np.arange(QCW)[None, :]
    for a in range(4):
        m[a] = (ql >= (P * a + kl)).astype(np.float32)

    in_maps = []
    for c in range(NCORES):
        e0 = c * E
        in_maps.append({
            "xT": xT,
            "wqT": np.ascontiguousarray(np.asarray(wq)[e0:e0 + E, :].T),
            "wkT": np.ascontiguousarray(np.asarray(wk)[e0:e0 + E, :].T),
            "wvT": np.ascontiguousarray(np.asarray(wv)[e0:e0 + E, :].T),
            "woT": np.ascontiguousarray(np.asarray(wo)[:, e0:e0 + E].T),
            "bqh": np.ascontiguousarray(
                np.asarray(bq)[e0:e0 + E].reshape(HPC, P)),
            "bkh": np.ascontiguousarray(
                np.asarray(bk)[e0:e0 + E].reshape(HPC, P)),
            "bvh": np.ascontiguousarray(
                np.asarray(bv)[e0:e0 + E].reshape(HPC, P)),
            "masks": m,
            "ones_d": np.ones((P, P), dtype=np.float32),
            "ident_d": np.eye(P, dtype=np.float32),
        })
    return in_maps


def kernel(**inputs):
    if "nc" not in _CACHE:
        _CACHE["nc"] = _build()
    nc = _CACHE["nc"]

    in_maps = _host_prep(
        inputs["x"], inputs["wq"], inputs["bq"], inputs["wk"], inputs["bk"],
        inputs["wv"], inputs["bv"], inputs["wo"],
    )

    res = bass_utils.run_bass_kernel_spmd(
        nc, in_maps, core_ids=list(range(NCORES)),
        trace=bool(os.environ.get("BASS_TRACE")),
    )
    _CACHE["last_result"] = res

    y = np.zeros((BN, C), dtype=np.float64)
    for c in range(NCORES):
        y += res.results[c]["yp"].astype(np.float64)
    y += np.asarray(inputs["bo"], dtype=np.float64)
    return y.astype(np.float32).reshape(B, N, C)


# revision 16
# speedup vs baseline: 3.4997x; 1.1560x over previous
"""Trainium2 Bass kernel: causal multi-head attention (B=2, N=2048, C=2048, 16 heads).

Sharding: 16 heads split across 8 cores (2 heads/core, tensor parallel).
Each core computes q/k/v projections for its 2 heads, causal attention,
and its partial out-projection y_c = ctx_c @ wo_c.T. Host sums partials + bo.

Per-core layout trick: everything is computed "transposed":
  qT/kT/vT: [head_dim(128) partitions, tokens]  (from lhsT=w^T, rhs=x^T)
  vT is PE-transposed back to V natural [tok, d] for the AV matmul
  S^T[k, q] = K^T.T @ Q^T tiles              (contraction over head_dim)
  E^T = exp(scale * S^T) (no max subtraction -- scores are ~N(0, 1/9))
  E tiles accumulated over k-tiles on DVE; a single all-ones matmul
  produces the row sums already broadcast across partitions; wide
  reciprocal + multiply normalizes ctx^T
  ctx^T[d, q] = V.T @ E^T  (lhsT = V natural [tok, d])
  y[tok, f] = ctx^T.T @ wo^T  (natural output layout)
Causality at tile granularity: only k-tiles with k <= q_max computed;
diagonal tiles multiplied by precomputed 0/1 masks after exp.
All matmul operands are float32r (full-rate PE streaming, ~19-bit mantissa).
Phase 1 walks c-tiles in the outer loop so x^T quarters release early and
the next chunk's DMA overlaps compute.
"""

import os
import numpy as np

import concourse.bass as bass
import concourse.tile as tile
from concourse import bacc, mybir
from concourse import bass_utils

F32 = mybir.dt.float32
F32R = mybir.dt.float32r
AF = mybir.ActivationFunctionType

# problem dims (hardcoded per contract)
B = 2
N = 2048
C = 2048
HEADS = 16
HD = 128          # head dim
NCORES = 8
HPC = HEADS // NCORES  # heads per core = 2
E = HPC * HD      # per-core projection width = 256
BN = B * N        # 4096
P = 128
CT = C // P       # 16 contraction tiles
NCH = 512         # n-chunk width for projections
NCHUNKS = BN // NCH   # 8
QCW = 512         # q-chunk width in attention
QCHUNKS = N // QCW    # 4 per batch
KT_PER_B = N // P     # 16 k-tiles per batch
TOK_TILES = BN // P   # 32
SCALE = float(HD) ** -0.5
XQ = 4            # x streamed in quarters of 4 c-tiles

_CACHE = {}


def _build():
    nc = bacc.Bacc(
        "TRN2",
        target_bir_lowering=False,
        debug=False,
        enable_asserts=False,
        num_devices=NCORES,
    )

    xT = nc.dram_tensor("xT", [C, BN], F32R, kind="ExternalInput").ap()
    wqT = nc.dram_tensor("wqT", [C, E], F32R, kind="ExternalInput").ap()
    wkT = nc.dram_tensor("wkT", [C, E], F32R, kind="ExternalInput").ap()
    wvT = nc.dram_tensor("wvT", [C, E], F32R, kind="ExternalInput").ap()
    woT = nc.dram_tensor("woT", [E, C], F32R, kind="ExternalInput").ap()
    bqh = nc.dram_tensor("bqh", [HPC, P], F32, kind="ExternalInput").ap()
    bkh = nc.dram_tensor("bkh", [HPC, P], F32, kind="ExternalInput").ap()
    bvh = nc.dram_tensor("bvh", [HPC, P], F32, kind="ExternalInput").ap()
    masks = nc.dram_tensor("masks", [4, P, QCW], F32R, kind="ExternalInput").ap()
    ones_d = nc.dram_tensor("ones_d", [P, P], F32R, kind="ExternalInput").ap()
    ident_d = nc.dram_tensor("ident_d", [P, P], F32, kind="ExternalInput").ap()
    yp = nc.dram_tensor("yp", [BN, C], F32, kind="ExternalOutput").ap()

    with tile.TileContext(nc) as tc:
        with tc.tile_pool(name="persist", bufs=1) as persist:
            # persistent per-core activations
            qT = persist.tile([P, HPC, B, N], F32R, tag="qT")
            kT = persist.tile([P, HPC, B, N], F32R, tag="kT")
            vN = persist.tile([P, TOK_TILES, E], F32R, tag="vN")
            masks_sb = persist.tile([P, 4, QCW], F32R, tag="masks")
            ones_sb = persist.tile([P, P], F32R, tag="ones")
            ident_sb = persist.tile([P, P], F32, tag="ident")

            # ---------------- Phase 1: projections ----------------
            with tc.tile_pool(name="p1w", bufs=1) as wpool, \
                 tc.tile_pool(name="p1x", bufs=4) as xpool, \
                 tc.tile_pool(name="p1vt", bufs=2) as vtpool, \
                 tc.tile_pool(name="p1_ps", bufs=6, space="PSUM") as pps, \
                 tc.tile_pool(name="p1t_ps", bufs=2, space="PSUM") as tps:
                wq_sb = wpool.tile([P, CT, E], F32R, tag="wq")
                wk_sb = wpool.tile([P, CT, E], F32R, tag="wk")
                wv_sb = wpool.tile([P, CT, E], F32R, tag="wv")
                bq_sb = wpool.tile([P, HPC], F32, tag="bq")
                bk_sb = wpool.tile([P, HPC], F32, tag="bk")
                bv_sb = wpool.tile([P, HPC], F32, tag="bv")

                # DMA priority: first weight pieces + chunk-0 x quarters
                # land first so the PE starts within a few us.
                nc.sync.dma_start(bq_sb[:], bqh.rearrange("h p -> p h"))
                nc.sync.dma_start(bk_sb[:], bkh.rearrange("h p -> p h"))
                nc.sync.dma_start(bv_sb[:], bvh.rearrange("h p -> p h"))
                wsrc = [(wq_sb, wqT), (wk_sb, wkT), (wv_sb, wvT)]
                for (dst, src) in wsrc:
                    srcr = src.rearrange("(t p) e -> p t e", p=P)
                    nc.sync.dma_start(dst[:, 0:4, :], srcr[:, 0:4, :])

                xTr = xT.rearrange("(t p) n -> p t n", p=P)
                xh0 = []
                for quarter in range(XQ):
                    xc = xpool.tile([P, CT // XQ, NCH], F32R, tag="xc")
                    nc.sync.dma_start(
                        xc[:], xTr[:, quarter * 4:(quarter + 1) * 4, 0:NCH])
                    xh0.append(xc)

                for (dst, src) in wsrc:
                    srcr = src.rearrange("(t p) e -> p t e", p=P)
                    for piece in range(1, 4):
                        nc.sync.dma_start(
                            dst[:, piece * 4:(piece + 1) * 4, :],
                            srcr[:, piece * 4:(piece + 1) * 4, :],
                        )
                nc.sync.dma_start(masks_sb[:], masks.rearrange("a p n -> p a n"))
                nc.sync.dma_start(ones_sb[:], ones_d)
                nc.sync.dma_start(ident_sb[:], ident_d)

                for ch in range(NCHUNKS):
                    b = ch // (N // NCH)
                    nn0 = (ch % (N // NCH)) * NCH  # within-batch token offset
                    n0 = ch * NCH                  # global token offset
                    if ch == 0:
                        xh = xh0
                    else:
                        xh = []
                        for quarter in range(XQ):
                            xc = xpool.tile([P, CT // XQ, NCH], F32R, tag="xc")
                            nc.sync.dma_start(
                                xc[:], xTr[:, quarter * 4:(quarter + 1) * 4,
                                           n0:n0 + NCH])
                            xh.append(xc)

                    # 6 accumulators (q/k/v x 2 heads); c-tile outer loop so
                    # each x quarter is released after its 4 c-tiles.
                    accs = [pps.tile([P, NCH], F32, tag="pacc",
                                     name=f"pacc_{ch}_{i}")
                            for i in range(3 * HPC)]
                    for ct in range(CT):
                        xq = xh[ct // XQ][:, ct % XQ, :]
                        for wi, (wsb, _) in enumerate(wsrc):
                            for h in range(HPC):
                                nc.tensor.matmul(
                                    accs[wi * HPC + h][:],
                                    wsb[:, ct, h * HD:(h + 1) * HD],
                                    xq,
                                    start=(ct == 0),
                                    stop=(ct == CT - 1),
                                )

                    for h in range(HPC):
                        nc.scalar.activation(
                            qT[:, h, b, nn0:nn0 + NCH], accs[h][:],
                            AF.Identity, bias=bq_sb[:, h:h + 1], scale=1.0)
                        nc.scalar.activation(
                            kT[:, h, b, nn0:nn0 + NCH], accs[HPC + h][:],
                            AF.Identity, bias=bk_sb[:, h:h + 1], scale=1.0)
                        # v^T with bias, then PE-transpose to V natural
                        vt = vtpool.tile([P, NCH], F32, tag="vt")
                        nc.scalar.activation(
                            vt[:], accs[2 * HPC + h][:],
                            AF.Identity, bias=bv_sb[:, h:h + 1], scale=1.0)
                        for ts in range(NCH // P):
                            tp = tps.tile([P, P], F32, tag="tp")
                            nc.tensor.transpose(
                                tp[:], vt[:, ts * P:(ts + 1) * P], ident_sb[:])
                            nc.vector.tensor_copy(
                                vN[:, ch * (NCH // P) + ts,
                                   h * HD:(h + 1) * HD],
                                tp[:])

            # ---------------- Phase 2: attention + out-proj ----------------
            with tc.tile_pool(name="p2const", bufs=1) as cpool, \
                 tc.tile_pool(name="p2e", bufs=4) as epool, \
                 tc.tile_pool(name="p2ctx", bufs=4) as ctxpool, \
                 tc.tile_pool(name="p2sm", bufs=3) as smpool, \
                 tc.tile_pool(name="p2y", bufs=2) as ysbpool, \
                 tc.tile_pool(name="p2s_ps", bufs=2, space="PSUM") as spool, \
                 tc.tile_pool(name="p2c_ps", bufs=2, space="PSUM") as cps, \
                 tc.tile_pool(name="p2sb_ps", bufs=2, space="PSUM") as sbps, \
                 tc.tile_pool(name="p2y_ps", bufs=2, space="PSUM") as yps:
                wo_sb = cpool.tile([P, HPC, C], F32R, tag="wo")
                nc.sync.dma_start(wo_sb[:], woT.rearrange("(h p) f -> p h f", p=P))

                for b in range(B):
                    for qc in range(QCHUNKS):
                        nkt = 4 * qc + 4  # causal: k-tiles 0..4qc+3
                        ctx_tiles = []
                        for h in range(HPC):
                            ctxu_ps = cps.tile([P, QCW], F32, tag="ctxu")
                            sums_bc = sbps.tile([P, QCW], F32, tag="sumbc")
                            for kt in range(nkt):
                                sps = spool.tile([P, QCW], F32, tag="s")
                                nc.tensor.matmul(
                                    sps[:],
                                    kT[:, h, b, kt * P:(kt + 1) * P],
                                    qT[:, h, b, qc * QCW:(qc + 1) * QCW],
                                    start=True, stop=True,
                                )
                                et = epool.tile([P, QCW], F32R, tag="e")
                                nc.scalar.activation(
                                    et[:], sps[:], AF.Exp, scale=SCALE
                                )
                                a = kt - 4 * qc
                                if a >= 0:  # diagonal tile: causal mask
                                    nc.vector.tensor_mul(
                                        et[:], et[:], masks_sb[:, a, :]
                                    )
                                nc.tensor.matmul(
                                    ctxu_ps[:],
                                    vN[:, b * KT_PER_B + kt, h * HD:(h + 1) * HD],
                                    et[:],
                                    start=(kt == 0), stop=(kt == nkt - 1),
                                )
                                # all-ones lhsT: rows of out = sums over
                                # k, i.e. reduce + broadcast in one matmul,
                                # accumulated across k-tiles in PSUM
                                nc.tensor.matmul(
                                    sums_bc[:], ones_sb[:], et[:],
                                    start=(kt == 0), stop=(kt == nkt - 1),
                                )
                            recip_bc = smpool.tile([P, QCW], F32, tag="recipbc")
                            nc.vector.reciprocal_approx_fast(recip_bc[:], sums_bc[:])
                            ctx = ctxpool.tile([P, QCW], F32R, tag="ctx")
                            nc.vector.tensor_mul(ctx[:], ctxu_ps[:], recip_bc[:])
                            ctx_tiles.append(ctx)

                        # out-projection for this (b, qc) block of tokens
                        for nt in range(QCW // P):
                            y_sb = ysbpool.tile([P, C], F32, tag="ysb")
                            for fc in range(C // 512):
                                y_ps = yps.tile([P, 512], F32, tag="yps")
                                for h in range(HPC):
                                    nc.tensor.matmul(
                                        y_ps[:],
                                        ctx_tiles[h][:, nt * P:(nt + 1) * P],
                                        wo_sb[:, h, fc * 512:(fc + 1) * 512],
                                        start=(h == 0), stop=(h == HPC - 1),
                                    )
                                nc.vector.tensor_copy(
                                    y_sb[:, fc * 512:(fc + 1) * 512], y_ps[:]
                                )
                            row0 = b * N + qc * QCW + nt * P
                            nc.sync.dma_start(yp[row0:row0 + P, :], y_sb[:])

    nc.compile()
    return nc


def _host_prep(x, wq, bq, wk, bk, wv, bv, wo):
    """Build the 8 per-core input maps."""
    x = np.asarray(x, dtype=np.float32)
    xT = np.ascontiguousarray(x.reshape(BN, C).T)  # [C, BN]

    m = np.zeros((4, P, QCW), dtype=np.float32)
    kl = np.arange(P)[:, None]
    ql = np.arange(QCW)[None, :]
    for a in range(4):
        m[a] = (ql >= (P * a + kl)).astype(np.float32)

    in_maps = []
    for c in range(NCORES):
        e0 = c * E
        in_maps.append({
            "xT": xT,
            "wqT": np.ascontiguousarray(np.asarray(wq)[e0:e0 + E, :].T),
            "wkT": np.ascontiguousarray(np.asarray(wk)[e0:e0 + E, :].T),
            "wvT": np.ascontiguousarray(np.asarray(wv)[e0:e0 + E, :].T),
            "woT": np.ascontiguousarray(np.asarray(wo)[:, e0:e0 + E].T),
            "bqh": np.ascontiguousarray(
                np.asarray(bq)[e0:e0 + E].reshape(HPC, P)),
            "bkh": np.ascontiguousarray(
                np.asarray(bk)[e0:e0 + E].reshape(HPC, P)),
            "bvh": np.ascontiguousarray(
                np.asarray(bv)[e0:e0 + E].reshape(HPC, P)),
            "masks": m,
            "ones_d": np.ones((P, P), dtype=np.float32),
            "ident_d": np.eye(P, dtype=np.float32),
        })
    return in_maps


def kernel(**inputs):
    if "nc" not in _CACHE:
        _CACHE["nc"] = _build()
    nc = _CACHE["nc"]

    in_maps = _host_prep(
        inputs["x"], inputs["wq"], inputs["bq"], inputs["wk"], inputs["bk"],
        inputs["wv"], inputs["bv"], inputs["wo"],
    )

    res = bass_utils.run_bass_kernel_spmd(
        nc, in_maps, core_ids=list(range(NCORES)),
        trace=bool(os.environ.get("BASS_TRACE")),
    )
    _CACHE["last_result"] = res

    y = np.zeros((BN, C), dtype=np.float64)
    for c in range(NCORES):
        y += res.results[c]["yp"].astype(np.float64)
    y += np.asarray(inputs["bo"], dtype=np.float64)
    return y.astype(np.float32).reshape(B, N, C)
